# revision 1
# baseline (speedup 1.0000x reference)
"""BiGNN message-passing kernel for Trainium2 (8 NeuronCores, Bass/Tile).

Reference computation (N=100000 nodes, E=600000 edges, D=128):
    msgs = vals[:, None] * features[cols]            # gather + scale
    x    = segment_sum(msgs, rows)                   # scatter-add to rows
    out  = (features + x) @ W1 + b1 + (x * features) @ W2 + b2

Sharding: destination nodes (rows) are sharded across the 8 cores, 12500
each; `features` is replicated into every core's HBM, so the per-edge
source gather is core-local (no collectives).

The kernel is DMA-byte-bound: all HWDGE rings and the 4 SWDGE gather
queues share the core's 16 DMA engines (~220 GB/s aggregate), so both
big per-edge streams run in float8_e3m4 (1.56% quantization step —
measured end-to-end rel err 1.7e-2 against the 2e-2 gate):

  * gathered source features G: the fp8 feature table is laid out as
    [cc, 2, 128] with the payload in [:, 0, :], so each 256B-stride row
    carries a 128B fp8 feature vector; dma_gather is emitted directly
    (InstDMAGatherAnt) because the bass wrapper asserts 256B elements
    while the SWDGE ucode handles any packet length with a 256B-aligned
    row stride.  128B/edge instead of 256B.
  * S blocks (one-hot x val): host-built fp8, streamed on the SP ring.

The segment-sum runs on TensorE as one matmul per 128-edge block:

    xT[f, d] += G_blk[e, f].T @ S_blk[e, d]       (fp8 x fp8 -> f32 psum)

Finished xT psums are evicted to SBUF in fp16 by the scalar engine, and
the dense epilogue for group g-1 is emitted inside group g so the
DVE/PE queues never stall on each other:

    outT = W1.T @ (fT + xT) + W2.T @ (xT * fT) + (b1 + b2)

featT / outT move in fp16 on the ACT ring; the host transposes and
upcasts per-core outputs back to fp32.
"""

import numpy as np

P = 128
D = 128
N_NODES = 100000
N_EDGES = 600000
N_CORES = 8
NCHUNKS = 4  # feature-table column chunks (int16 index reach)
GROUP_TILES = 8  # dest tiles per gather/store group

_LAST_RESULTS = None  # BassKernelResults of the most recent run (for test.py)


def _prep(rows, cols, vals, n_nodes, n_cores):
    """Host-side edge reorganization into the shared block schedule.

    Returns (sched, per_core):
      sched: tiles/npc/cc/groups/tile_blocks/NB/TOT
      per_core[c]: idx16 [128, TOT/16] int16, S8 [128, NB*128] fp8e3m4
    """
    import ml_dtypes

    npc = n_nodes // n_cores
    tiles = (npc + P - 1) // P
    cc = n_nodes // NCHUNKS
    assert n_nodes % NCHUNKS == 0

    rows = np.asarray(rows, dtype=np.int64)
    cols = np.asarray(cols, dtype=np.int64)
    vals = np.asarray(vals, dtype=np.float32)
    e = rows.shape[0]

    core = rows // npc
    local = rows - core * npc
    tile_idx = local // P
    dest_in_tile = (local - tile_idx * P).astype(np.int64)
    j_idx = cols // cc

    key = (core * tiles + tile_idx) * NCHUNKS + j_idx
    order = np.argsort(key, kind="stable")
    cols_s = cols[order]
    dest_s = dest_in_tile[order]
    vals_s = vals[order]
    j_s = j_idx[order]

    cnt = np.bincount(key[order], minlength=n_cores * tiles * NCHUNKS).reshape(
        n_cores, tiles, NCHUNKS
    )
    starts_flat = np.concatenate([[0], np.cumsum(cnt.reshape(-1))[:-1]])
    rank = np.arange(e) - np.repeat(starts_flat, cnt.reshape(-1))

    B = (cnt.max(axis=0) + P - 1) // P  # blocks per (tile, j), shared
    empty = B.sum(axis=1) == 0
    B[empty, 0] = 1

    groups = []
    blk_of_tj = np.zeros((tiles, NCHUNKS), dtype=np.int64)
    nb = 0
    for g0 in range(0, tiles, GROUP_TILES):
        g1 = min(g0 + GROUP_TILES, tiles)
        sections = []
        for j in range(NCHUNKS):
            sec_start = nb
            for t in range(g0, g1):
                blk_of_tj[t, j] = nb
                nb += B[t, j]
            sections.append((sec_start, nb - sec_start))
        groups.append((g0, g1, sections))
    NB = nb
    TOT = NB * P

    tile_blocks = []
    for t in range(tiles):
        lst = []
        for j in range(NCHUNKS):
            for b in range(B[t, j]):
                lst.append((int(blk_of_tj[t, j] + b), j))
        tile_blocks.append(lst)

    slot_s = blk_of_tj[tile_idx[order], j_s] * P + rank

    per_core = []
    core_s = core[order]
    for c in range(n_cores):
        m = core_s == c
        s = slot_s[m]
        idx_flat = np.zeros(TOT, dtype=np.int16)
        idx_flat[s] = (cols_s[m] - j_s[m] * cc).astype(np.int16)
        idx16 = np.tile(np.ascontiguousarray(idx_flat.reshape(-1, 16).T), (8, 1))
        # per-slot payload (partition = slot%128, free = block); padded
        # slots keep val=0 so their S rows are zero
        dest_flat = np.zeros((NB, P), dtype=np.float16)
        val_flat = np.zeros((NB, P), dtype=np.float16)
        dest_flat[s // P, s % P] = dest_s[m].astype(np.float16)
        val_flat[s // P, s % P] = vals_s[m].astype(np.float16)
        per_core.append(
            {
                "idx16": np.ascontiguousarray(idx16),
                "dest16": np.ascontiguousarray(dest_flat.T),
                "val16": np.ascontiguousarray(val_flat.T),
            }
        )

    sched = {
        "tiles": tiles,
        "npc": npc,
        "cc": cc,
        "groups": groups,
        "tile_blocks": tile_blocks,
        "NB": NB,
        "TOT": TOT,
    }
    return sched, per_core


def _raw_gather_128(eng, mybir, out_ap, in_ap, idxs_ap, num_idxs, queue_num):
    """dma_gather with a 128-byte element on a 256-byte-stride table.

    Mirrors bass's dma_gather (non-transpose, DRAM source, no prepare)
    but skips its 256B-element assert: the SWDGE ucode packetizes any
    elem_size (packet = min(elem_size_bytes, 16K)); only the row stride
    must be a 256B multiple (stride_bytes_256 field).
    """
    eng._assert_queue_num(queue_num)
    elem_size = 128  # fp8 elements = 128 bytes
    elem_step = 256  # table row stride in fp8 elements = 256 bytes
    assert in_ap.ap[0][0] == elem_step, in_ap.ap
    assert in_ap.ap[-1][1] == elem_size, in_ap.ap
    assert out_ap.ap[-1][1] == elem_size, out_ap.ap
    assert out_ap.ap[0][1] * out_ap.ap[1][1] == num_idxs, out_ap.ap
    _in_ap = eng.lower_ap_dma(in_ap, for_custom_bir_dma=True)
    _idxs_ap = eng.lower_ap(idxs_ap)
    _out_ap = eng.lower_ap(out_ap)
    return eng.add_instruction(
        mybir.InstDMAGatherAnt(
            name=eng.bass.get_next_instruction_name(),
            ins=[
                *_in_ap,
                _idxs_ap,
                eng.lower_val_access(eng.to_reg(num_idxs)),
            ],
            outs=[_out_ap],
            transpose=False,
            num_idxs=num_idxs,
            elem_size=elem_size,
            stride_bytes_256=1,
            gen_mode=0,
            single_packet=False,
            queue_num=queue_num,
            sbuf_tokens_per_rank=0,
            sbuf_free_dim_per_rank=0,
            sbuf_free_dim_pad_per_rank=0,
            sbuf_byte_offset=0,
        )
    )


def _build_program(n_nodes, sched):
    import concourse.bacc as bacc
    import concourse.mybir as mybir
    import concourse.tile as tile

    f32 = mybir.dt.float32
    f16 = mybir.dt.float16
    f8 = mybir.dt.float8e3
    i16 = mybir.dt.int16

    npc = sched["npc"]
    cc = sched["cc"]
    NB = sched["NB"]
    TOT = sched["TOT"]
    tile_blocks = sched["tile_blocks"]

    nc = bacc.Bacc(num_swdge_queues=4)
    feat8 = [
        nc.dram_tensor(f"feat8_{j}", [cc, 2, P], f8, kind="ExternalInput")
        for j in range(NCHUNKS)
    ]
    featT = nc.dram_tensor("featT", [D, npc], f16, kind="ExternalInput")
    w1 = nc.dram_tensor("W1", [D, D], f16, kind="ExternalInput")
    w2 = nc.dram_tensor("W2", [D, D], f16, kind="ExternalInput")
    bsum = nc.dram_tensor("bsum", [D, 2], f32, kind="ExternalInput")
    idx16 = nc.dram_tensor("idx16", [P, TOT // 16], i16, kind="ExternalInput")
    dest16 = nc.dram_tensor("dest16", [P, NB], f16, kind="ExternalInput")
    val16 = nc.dram_tensor("val16", [P, NB], f16, kind="ExternalInput")
    iota16 = nc.dram_tensor("iota16", [P, P], f16, kind="ExternalInput")
    outT = nc.dram_tensor("outT", [D, npc], f16, kind="ExternalOutput")

    with tile.TileContext(nc) as tc:
        with (
            tc.tile_pool(name="const", bufs=1) as constp,
            tc.tile_pool(name="gpool", bufs=5) as gpool,
            tc.tile_pool(name="spool", bufs=8) as spool,
            tc.tile_pool(name="ftpool", bufs=4) as ftpool,
            tc.tile_pool(name="xspool", bufs=4) as xspool,
            tc.tile_pool(name="ampool", bufs=6) as ampool,
            tc.tile_pool(name="ostage", bufs=3) as ostagep,
            tc.tile_pool(name="psx", bufs=6, space="PSUM") as psx,
            tc.tile_pool(name="pso", bufs=2, space="PSUM") as pso,
        ):
            # --- constants (idx16 first: every gather depends on it) ---
            idx16_t = constp.tile([P, TOT // 16], i16)
            nc.scalar.dma_start(out=idx16_t[:], in_=idx16[:, :])
            iota_t = constp.tile([P, P], f16)
            nc.sync.dma_start(out=iota_t[:], in_=iota16[:, :])
            # group 0's dest/val slices load first so the first S-build
            # starts ~1us in instead of behind the full 0.45MB payload
            n0 = sched["groups"][0][2][-1][0] + sched["groups"][0][2][-1][1]
            dest_t = constp.tile([P, NB], f16)
            val_t = constp.tile([P, NB], f16)
            nc.sync.dma_start(out=dest_t[:, :n0], in_=dest16[:, :n0])
            nc.sync.dma_start(out=val_t[:, :n0], in_=val16[:, :n0])
            nc.sync.dma_start(out=dest_t[:, n0:], in_=dest16[:, n0:])
            nc.sync.dma_start(out=val_t[:, n0:], in_=val16[:, n0:])
            w1_t = constp.tile([P, P], f16)
            nc.sync.dma_start(out=w1_t[:], in_=w1[:, :])
            w2_t = constp.tile([P, P], f16)
            nc.sync.dma_start(out=w2_t[:], in_=w2[:, :])
            bias_t = constp.tile([P, 2], f32)
            nc.sync.dma_start(out=bias_t[:], in_=bsum[:, :])

            def emit_epilogue(g0, gw, fT, xS, oT):
                aT = ampool.tile([P, gw], f16, tag="aT")
                mT = ampool.tile([P, gw], f16, tag="mT")
                nc.vector.tensor_tensor(
                    out=aT[:], in0=xS[:, :gw], in1=fT[:, :gw],
                    op=mybir.AluOpType.add,
                )
                nc.vector.tensor_tensor(
                    out=mT[:], in0=xS[:, :gw], in1=fT[:, :gw],
                    op=mybir.AluOpType.mult,
                )
                for c0 in range(0, gw, 512):
                    cw = min(512, gw - c0)
                    out2 = pso.tile([P, 512], f32, tag="out2")
                    nc.tensor.matmul(
                        out=out2[:, :cw], lhsT=w1_t[:], rhs=aT[:, c0 : c0 + cw],
                        start=True, stop=False,
                    )
                    nc.tensor.matmul(
                        out=out2[:, :cw], lhsT=w2_t[:], rhs=mT[:, c0 : c0 + cw],
                        start=False, stop=True,
                    )
                    nc.scalar.activation(
                        out=oT[:, c0 : c0 + cw],
                        in_=out2[:, :cw],
                        func=mybir.ActivationFunctionType.Identity,
                        bias=bias_t[:, 0:1],
                        scale=1.0,
                    )
                nc.scalar.dma_start(
                    out=outT[:, g0 * P : g0 * P + gw], in_=oT[:, :gw]
                )

            prev = None
            for g0, g1, sections in sched["groups"]:
                gw = min(g1 * P, npc) - g0 * P
                ch0 = sections[0][0]
                ch1 = sections[-1][0] + sections[-1][1]
                nch = ch1 - ch0

                # one dma_gather per feature-table chunk, parallel SWDGE
                # queues; 128B fp8 payload per edge from 256B-stride rows
                gtiles = {}
                for j in range(NCHUNKS):
                    sec_start, sec_nblk = sections[j]
                    if sec_nblk == 0:
                        continue
                    G = gpool.tile([P, sec_nblk, P], f8, tag=f"G{j}")
                    n_idx = sec_nblk * P
                    _raw_gather_128(
                        nc.gpsimd,
                        mybir,
                        G[:],
                        feat8[j][:, 0, :],
                        idx16_t[:, sec_start * 8 : sec_start * 8 + n_idx // 16],
                        n_idx,
                        queue_num=j,
                    )
                    gtiles[j] = (G, sec_start)

                # S blocks built on DVE from the per-slot payload: two
                # whole-group broadcast tensor_tensor passes
                #   S[e,b,d] = (iota[d] == dest[e,b]) * val[e,b]
                S = spool.tile([P, nch, P], f8, tag="S")
                nc.vector.tensor_tensor(
                    out=S[:],
                    in0=iota_t[:, None, :].broadcast_to([P, nch, P]),
                    in1=dest_t[:, ch0:ch1, None].broadcast_to([P, nch, P]),
                    op=mybir.AluOpType.is_equal,
                )
                nc.vector.tensor_tensor(
                    out=S[:],
                    in0=S[:],
                    in1=val_t[:, ch0:ch1, None].broadcast_to([P, nch, P]),
                    op=mybir.AluOpType.mult,
                )

                # featT slice for this group, on the ACT HWDGE ring
                fT = ftpool.tile([P, gw], f16, tag="fT")
                nc.scalar.dma_start(out=fT[:], in_=featT[:, g0 * P : g0 * P + gw])

                # xT psum per tile; evict to fp16 SBUF on the scalar engine
                xS = xspool.tile([P, gw], f16, tag="xS")
                for t in range(g0, g1):
                    w = min((t + 1) * P, npc) - t * P
                    blocks = tile_blocks[t]
                    xT = psx.tile([P, P], f32, tag="xT")
                    for bi, (blk, j) in enumerate(blocks):
                        G, sec_start = gtiles[j]
                        nc.tensor.matmul(
                            out=xT[:],
                            lhsT=G[:, blk - sec_start, :],
                            rhs=S[:, blk - ch0, :],
                            start=(bi == 0),
                            stop=(bi == len(blocks) - 1),
                        )
                    toff = (t - g0) * P
                    nc.scalar.activation(
                        out=xS[:, toff : toff + w],
                        in_=xT[:, :w],
                        func=mybir.ActivationFunctionType.Identity,
                        bias=bias_t[:, 1:2],
                        scale=1.0,
                    )

                oT = ostagep.tile([P, gw], f16, tag="oT")

                # dense epilogue for the PREVIOUS group: keeps the DVE queue
                # ordered [am(g-1) after xS(g) exists] so PE(g) never waits.
                if prev is not None:
                    emit_epilogue(*prev)
                prev = (g0, gw, fT, xS, oT)

            emit_epilogue(*prev)
    nc.compile()
    return nc


def _run(rows, cols, vals, features, W1, b1, W2, b2, n_nodes, n_cores):
    global _LAST_RESULTS
    import ml_dtypes
    from concourse import bass_utils

    npc = n_nodes // n_cores
    features = np.ascontiguousarray(np.asarray(features, dtype=np.float32))
    W1_16 = np.ascontiguousarray(np.asarray(W1, dtype=np.float32).astype(np.float16))
    W2_16 = np.ascontiguousarray(np.asarray(W2, dtype=np.float32).astype(np.float16))
    bsum = np.zeros((D, 2), dtype=np.float32)
    bsum[:, 0] = np.asarray(b1, dtype=np.float32) + np.asarray(b2, dtype=np.float32)
    bsum = np.ascontiguousarray(bsum)
    iota = np.ascontiguousarray(
        np.tile(np.arange(P, dtype=np.float16)[None, :], (P, 1))
    )

    sched, per_core = _prep(rows, cols, vals, n_nodes, n_cores)
    nc = _build_program(n_nodes, sched)

    cc = sched["cc"]
    feat8_flat = features.astype(np.float16).astype(ml_dtypes.float8_e3m4)
    feat8_chunks = []
    for j in range(NCHUNKS):
        chunk = np.zeros((cc, 2, P), dtype=ml_dtypes.float8_e3m4)
        chunk[:, 0, :] = feat8_flat[j * cc : (j + 1) * cc, :]
        feat8_chunks.append(np.ascontiguousarray(chunk))

    in_maps = []
    for c in range(n_cores):
        featT_c = np.ascontiguousarray(
            features[c * npc : (c + 1) * npc, :].T.astype(np.float16)
        )
        im = {
            "featT": featT_c,
            "W1": W1_16,
            "W2": W2_16,
            "bsum": bsum,
            "idx16": per_core[c]["idx16"],
            "dest16": per_core[c]["dest16"],
            "val16": per_core[c]["val16"],
            "iota16": iota,
        }
        for j in range(NCHUNKS):
            im[f"feat8_{j}"] = feat8_chunks[j]
        in_maps.append(im)

    res = bass_utils.run_bass_kernel_spmd(nc, in_maps, core_ids=list(range(n_cores)))
    _LAST_RESULTS = res
    out = np.concatenate(
        [r["outT"].T.astype(np.float32) for r in res.results], axis=0
    )
    return np.ascontiguousarray(out)


def kernel(rows, cols, vals, features, W1, b1, W2, b2):
    return _run(rows, cols, vals, features, W1, b1, W2, b2, N_NODES, N_CORES)



# revision 2
# speedup vs baseline: 1.3294x; 1.3294x over previous
"""BiGNN message-passing kernel for Trainium2 (8 NeuronCores, Bass/Tile).

Reference computation (N=100000 nodes, E=600000 edges, D=128):
    msgs = vals[:, None] * features[cols]            # gather + scale
    x    = segment_sum(msgs, rows)                   # scatter-add to rows
    out  = (features + x) @ W1 + b1 + (x * features) @ W2 + b2

Sharding: destination nodes (rows) are sharded across the 8 cores, 12500
each; `features` is replicated into every core's HBM, so the per-edge
source gather is core-local (no collectives).

The critical path is GPSIMD (SWDGE) descriptor generation for the
per-edge feature gather: ~2.3 ns/index, strictly serialized on the one
POOL engine (each InstDMAGatherAnt activates only the Q7 core pair of
its queue).  Everything else is arranged to hide underneath it:

  * gathered source features G: fp8e3m4 table laid out [cc+1, 2, 128]
    (payload in [:, 0, :], row cc all-zero for padding slots), gathered
    as 128B elements from 256B-stride rows via a raw InstDMAGatherAnt.
  * edge slots are packed DENSELY per (group, chunk) section, sorted by
    destination tile: blocks of 128 slots may straddle tile boundaries,
    cutting gather padding from 25% to ~8%.  The matmul schedule is the
    UNION over the 8 cores of (block, tile) pieces; a core lacking a
    piece gets an all-zero S block there (contributes nothing).
  * the one-hot scatter matrices S (S[slot, dst] = val, one 128x128 fp8
    block per piece) are built on the HOST and STREAMED from HBM on the
    SP HWDGE ring instead of being built on DVE (which used to be a
    second ~235us serial bottleneck contending with GPSIMD for the
    shared POOL SBUF port).

The segment-sum runs on TensorE, one matmul per piece:

    xT[f, d] += G[e, f].T @ S[e, d]        (fp8 x fp8 -> f32 psum)

Finished xT psums are evicted to SBUF in fp16 by the scalar engine, and
the dense epilogue for group g-1 is emitted inside group g:

    outT = W1.T @ (fT + xT) + W2.T @ (xT * fT) + (b1 + b2)

featT / outT move in fp16 on the ACT ring; the host transposes and
upcasts per-core outputs back to fp32.
"""

import numpy as np

P = 128
D = 128
N_NODES = 100000
N_EDGES = 600000
N_CORES = 8
NCHUNKS = 4  # feature-table column chunks (int16 index reach)
GROUP_TILES = 8  # dest tiles per gather/store group

_LAST_RESULTS = None  # BassKernelResults of the most recent run (for test.py)


def _prep(rows, cols, vals, n_nodes, n_cores):
    """Host-side edge reorganization into the shared block schedule.

    Returns (sched, per_core):
      sched:
        tiles/npc/cc/ngroups/TOT/NP plus per-group gather sections and
        the shared matmul piece schedule (union over cores).
      per_core[c]:
        idx16 [128, TOT/16] int16   gather indices (pad -> zero row cc)
        S8    [128, NP*128] fp8e3m4 one-hot*val scatter blocks
    """
    import ml_dtypes

    npc = n_nodes // n_cores
    tiles = (npc + P - 1) // P
    ngroups = (tiles + GROUP_TILES - 1) // GROUP_TILES
    nsec = ngroups * NCHUNKS
    cc = n_nodes // NCHUNKS
    assert n_nodes % NCHUNKS == 0

    rows = np.asarray(rows, dtype=np.int64)
    cols = np.asarray(cols, dtype=np.int64)
    vals = np.asarray(vals, dtype=np.float32)

    core = rows // npc
    local = rows - core * npc
    t_all = local // P
    dit_all = (local - t_all * P).astype(np.int64)
    j_all = cols // cc
    sec_all = (t_all // GROUP_TILES) * NCHUNKS + j_all

    # shared per-section block counts (max over cores)
    cnt = np.zeros((n_cores, nsec), dtype=np.int64)
    for c in range(n_cores):
        cnt[c] = np.bincount(sec_all[core == c], minlength=nsec)
    nblk = (cnt.max(axis=0) + P - 1) // P
    nblk = np.maximum(nblk, 1)
    blk_base = np.concatenate([[0], np.cumsum(nblk)[:-1]])
    NBg = int(nblk.sum())
    TOT = NBg * P

    # per-core slot packing + piece keys
    per_core_raw = []
    union_pk = set()
    for c in range(n_cores):
        m = core == c
        sc = sec_all[m]
        tc = t_all[m]
        dc = dit_all[m]
        vc = vals[m]
        colc = (cols[m] - j_all[m] * cc).astype(np.int16)
        o = np.lexsort((tc, sc))
        sc, tc, dc, vc, colc = sc[o], tc[o], dc[o], vc[o], colc[o]
        starts = np.concatenate([[0], np.cumsum(cnt[c])[:-1]])
        rank = np.arange(sc.size) - starts[sc]
        slot = blk_base[sc] * P + rank
        babs = blk_base[sc] + rank // P
        pk = babs * P + tc  # tile index < 128
        union_pk.update(np.unique(pk).tolist())
        per_core_raw.append((slot, babs, tc, dc, vc, colc, pk))

    # shared piece schedule in emission order (tile asc, then block asc)
    pk_u = np.array(sorted(union_pk), dtype=np.int64)
    babs_u = pk_u // P
    tl_u = pk_u % P
    order = np.lexsort((babs_u, tl_u))
    NP = pk_u.size
    mb_of_rank = np.empty(NP, dtype=np.int64)  # rank in pk_u -> mb
    mb_of_rank[order] = np.arange(NP)

    sec_of_blk = np.repeat(np.arange(nsec), nblk)
    pieces_mb_sorted = np.empty(NP, dtype=np.int64)
    pieces_mb_sorted[:] = np.arange(NP)
    # emission-order piece attributes
    e_babs = babs_u[order]
    e_tl = tl_u[order]
    e_j = sec_of_blk[e_babs] % NCHUNKS

    # per-tile first/last piece flags
    tile_first = np.zeros(NP, dtype=bool)
    tile_last = np.zeros(NP, dtype=bool)
    tile_first[0] = True
    for i in range(1, NP):
        if e_tl[i] != e_tl[i - 1]:
            tile_first[i] = True
            tile_last[i - 1] = True
    tile_last[NP - 1] = True

    groups = []
    for g in range(ngroups):
        g0 = g * GROUP_TILES
        g1 = min(g0 + GROUP_TILES, tiles)
        sections = []
        for j in range(NCHUNKS):
            s = g * NCHUNKS + j
            sections.append((int(blk_base[s]), int(nblk[s])))
        in_g = (e_tl >= g0) & (e_tl < g1)
        mbs = np.nonzero(in_g)[0]
        ms0, ms1 = int(mbs.min()), int(mbs.max()) + 1
        tile_pieces = []
        for t in range(g0, g1):
            sel = np.nonzero(e_tl == t)[0]
            tile_pieces.append(
                [
                    (int(mb), int(e_babs[mb]), int(e_j[mb]),
                     bool(tile_first[mb]), bool(tile_last[mb]))
                    for mb in sel
                ]
            )
        groups.append((g0, g1, sections, ms0, ms1, tile_pieces))

    # per-core payloads
    per_core = []
    for c in range(n_cores):
        slot, babs, tc, dc, vc, colc, pk = per_core_raw[c]
        idx_flat = np.full(TOT, cc, dtype=np.int16)  # pad -> zero row
        idx_flat[slot] = colc
        idx16 = np.tile(np.ascontiguousarray(idx_flat.reshape(-1, 16).T), (8, 1))
        # edge -> emission mb
        pos = np.searchsorted(pk_u, pk)
        mb_e = mb_of_rank[pos]
        S8 = np.zeros((P, NP, P), dtype=ml_dtypes.float8_e3m4)
        S8[slot % P, mb_e, dc] = vc.astype(np.float16)
        per_core.append(
            {
                "idx16": np.ascontiguousarray(idx16),
                "S8": np.ascontiguousarray(S8.reshape(P, NP * P)),
            }
        )

    sched = {
        "tiles": tiles,
        "npc": npc,
        "cc": cc,
        "groups": groups,
        "NBg": NBg,
        "TOT": TOT,
        "NP": NP,
    }
    return sched, per_core


def _raw_gather_128(eng, mybir, out_ap, in_ap, idxs_ap, num_idxs, queue_num):
    """dma_gather with a 128-byte element on a 256-byte-stride table.

    Mirrors bass's dma_gather (non-transpose, DRAM source, no prepare)
    but skips its 256B-element assert: the SWDGE ucode packetizes any
    elem_size (packet = min(elem_size_bytes, 16K)); only the row stride
    must be a 256B multiple (stride_bytes_256 field).
    """
    eng._assert_queue_num(queue_num)
    elem_size = 128  # fp8 elements = 128 bytes
    elem_step = 256  # table row stride in fp8 elements = 256 bytes
    assert in_ap.ap[0][0] == elem_step, in_ap.ap
    assert in_ap.ap[-1][1] == elem_size, in_ap.ap
    assert out_ap.ap[-1][1] == elem_size, out_ap.ap
    assert out_ap.ap[0][1] * out_ap.ap[1][1] == num_idxs, out_ap.ap
    _in_ap = eng.lower_ap_dma(in_ap, for_custom_bir_dma=True)
    _idxs_ap = eng.lower_ap(idxs_ap)
    _out_ap = eng.lower_ap(out_ap)
    return eng.add_instruction(
        mybir.InstDMAGatherAnt(
            name=eng.bass.get_next_instruction_name(),
            ins=[
                *_in_ap,
                _idxs_ap,
                eng.lower_val_access(eng.to_reg(num_idxs)),
            ],
            outs=[_out_ap],
            transpose=False,
            num_idxs=num_idxs,
            elem_size=elem_size,
            stride_bytes_256=1,
            gen_mode=0,
            single_packet=False,
            queue_num=queue_num,
            sbuf_tokens_per_rank=0,
            sbuf_free_dim_per_rank=0,
            sbuf_free_dim_pad_per_rank=0,
            sbuf_byte_offset=0,
        )
    )


def _build_program(n_nodes, sched):
    import concourse.bacc as bacc
    import concourse.mybir as mybir
    import concourse.tile as tile

    f32 = mybir.dt.float32
    f16 = mybir.dt.float16
    f8 = mybir.dt.float8e3
    i16 = mybir.dt.int16

    npc = sched["npc"]
    cc = sched["cc"]
    TOT = sched["TOT"]
    NP = sched["NP"]

    nc = bacc.Bacc(num_swdge_queues=4)
    feat8 = [
        nc.dram_tensor(f"feat8_{j}", [cc + 1, 2, P], f8, kind="ExternalInput")
        for j in range(NCHUNKS)
    ]
    featT = nc.dram_tensor("featT", [D, npc], f16, kind="ExternalInput")
    w1 = nc.dram_tensor("W1", [D, D], f16, kind="ExternalInput")
    w2 = nc.dram_tensor("W2", [D, D], f16, kind="ExternalInput")
    bsum = nc.dram_tensor("bsum", [D, 2], f32, kind="ExternalInput")
    idx16 = nc.dram_tensor("idx16", [P, TOT // 16], i16, kind="ExternalInput")
    s8d = nc.dram_tensor("S8", [P, NP * P], f8, kind="ExternalInput")
    outT = nc.dram_tensor("outT", [D, npc], f16, kind="ExternalOutput")

    with tile.TileContext(nc) as tc:
        with (
            tc.tile_pool(name="const", bufs=1) as constp,
            tc.tile_pool(name="gpool", bufs=8) as gpool,
            tc.tile_pool(name="spool", bufs=3) as spool,
            tc.tile_pool(name="ftpool", bufs=4) as ftpool,
            tc.tile_pool(name="xspool", bufs=4) as xspool,
            tc.tile_pool(name="ampool", bufs=6) as ampool,
            tc.tile_pool(name="ostage", bufs=3) as ostagep,
            tc.tile_pool(name="psx", bufs=6, space="PSUM") as psx,
            tc.tile_pool(name="pso", bufs=2, space="PSUM") as pso,
        ):
            # --- constants (idx16 first: every gather depends on it).
            # group 0's slice loads first so the first gather starts
            # early instead of behind the full idx payload.
            n0 = sched["groups"][0][2][-1][0] + sched["groups"][0][2][-1][1]
            idx16_t = constp.tile([P, TOT // 16], i16)
            nc.scalar.dma_start(out=idx16_t[:, : n0 * 8], in_=idx16[:, : n0 * 8])
            nc.scalar.dma_start(out=idx16_t[:, n0 * 8 :], in_=idx16[:, n0 * 8 :])
            w1_t = constp.tile([P, P], f16)
            nc.sync.dma_start(out=w1_t[:], in_=w1[:, :])
            w2_t = constp.tile([P, P], f16)
            nc.sync.dma_start(out=w2_t[:], in_=w2[:, :])
            bias_t = constp.tile([P, 2], f32)
            nc.sync.dma_start(out=bias_t[:], in_=bsum[:, :])

            def emit_epilogue(g0, gw, fT, xS, oT):
                aT = ampool.tile([P, gw], f16, tag="aT")
                mT = ampool.tile([P, gw], f16, tag="mT")
                nc.vector.tensor_tensor(
                    out=aT[:], in0=xS[:, :gw], in1=fT[:, :gw],
                    op=mybir.AluOpType.add,
                )
                nc.vector.tensor_tensor(
                    out=mT[:], in0=xS[:, :gw], in1=fT[:, :gw],
                    op=mybir.AluOpType.mult,
                )
                for c0 in range(0, gw, 512):
                    cw = min(512, gw - c0)
                    out2 = pso.tile([P, 512], f32, tag="out2")
                    nc.tensor.matmul(
                        out=out2[:, :cw], lhsT=w1_t[:], rhs=aT[:, c0 : c0 + cw],
                        start=True, stop=False,
                    )
                    nc.tensor.matmul(
                        out=out2[:, :cw], lhsT=w2_t[:], rhs=mT[:, c0 : c0 + cw],
                        start=False, stop=True,
                    )
                    nc.scalar.activation(
                        out=oT[:, c0 : c0 + cw],
                        in_=out2[:, :cw],
                        func=mybir.ActivationFunctionType.Identity,
                        bias=bias_t[:, 0:1],
                        scale=1.0,
                    )
                nc.scalar.dma_start(
                    out=outT[:, g0 * P : g0 * P + gw], in_=oT[:, :gw]
                )

            prev = None
            for g0, g1, sections, ms0, ms1, tile_pieces in sched["groups"]:
                gw = min(g1 * P, npc) - g0 * P
                npg = ms1 - ms0

                # one dma_gather per feature-table chunk, parallel SWDGE
                # queues; 128B fp8 payload per edge from 256B-stride rows
                gtiles = {}
                for j in range(NCHUNKS):
                    sec_start, sec_nblk = sections[j]
                    G = gpool.tile([P, sec_nblk, P], f8, tag=f"G{j}")
                    n_idx = sec_nblk * P
                    _raw_gather_128(
                        nc.gpsimd,
                        mybir,
                        G[:],
                        feat8[j][:, 0, :],
                        idx16_t[:, sec_start * 8 : sec_start * 8 + n_idx // 16],
                        n_idx,
                        queue_num=j,
                    )
                    gtiles[j] = (G, sec_start)

                # host-built one-hot*val S blocks, streamed on SP ring
                S = spool.tile([P, npg, P], f8, tag="S")
                nc.sync.dma_start(out=S[:], in_=s8d[:, ms0 * P : ms1 * P])

                # featT slice for this group, on the ACT HWDGE ring
                fT = ftpool.tile([P, gw], f16, tag="fT")
                nc.scalar.dma_start(out=fT[:], in_=featT[:, g0 * P : g0 * P + gw])

                # xT psum per tile; evict to fp16 SBUF on the scalar engine
                xS = xspool.tile([P, gw], f16, tag="xS")
                for t in range(g0, g1):
                    w = min((t + 1) * P, npc) - t * P
                    pieces = tile_pieces[t - g0]
                    xT = psx.tile([P, P], f32, tag="xT")
                    for mb, babs, j, st, sp in pieces:
                        G, sec_start = gtiles[j]
                        nc.tensor.matmul(
                            out=xT[:],
                            lhsT=G[:, babs - sec_start, :],
                            rhs=S[:, mb - ms0, :],
                            start=st,
                            stop=sp,
                        )
                    toff = (t - g0) * P
                    nc.scalar.activation(
                        out=xS[:, toff : toff + w],
                        in_=xT[:, :w],
                        func=mybir.ActivationFunctionType.Identity,
                        bias=bias_t[:, 1:2],
                        scale=1.0,
                    )

                oT = ostagep.tile([P, gw], f16, tag="oT")

                # dense epilogue for the PREVIOUS group: keeps the DVE queue
                # ordered [am(g-1) after xS(g) exists] so PE(g) never waits.
                if prev is not None:
                    emit_epilogue(*prev)
                prev = (g0, gw, fT, xS, oT)

            emit_epilogue(*prev)
    nc.compile()
    return nc


def _run(rows, cols, vals, features, W1, b1, W2, b2, n_nodes, n_cores):
    global _LAST_RESULTS
    import ml_dtypes
    from concourse import bass_utils

    npc = n_nodes // n_cores
    features = np.ascontiguousarray(np.asarray(features, dtype=np.float32))
    W1_16 = np.ascontiguousarray(np.asarray(W1, dtype=np.float32).astype(np.float16))
    W2_16 = np.ascontiguousarray(np.asarray(W2, dtype=np.float32).astype(np.float16))
    bsum = np.zeros((D, 2), dtype=np.float32)
    bsum[:, 0] = np.asarray(b1, dtype=np.float32) + np.asarray(b2, dtype=np.float32)
    bsum = np.ascontiguousarray(bsum)

    sched, per_core = _prep(rows, cols, vals, n_nodes, n_cores)
    nc = _build_program(n_nodes, sched)

    cc = sched["cc"]
    feat8_flat = features.astype(np.float16).astype(ml_dtypes.float8_e3m4)
    feat8_chunks = []
    for j in range(NCHUNKS):
        chunk = np.zeros((cc + 1, 2, P), dtype=ml_dtypes.float8_e3m4)
        chunk[:cc, 0, :] = feat8_flat[j * cc : (j + 1) * cc, :]
        feat8_chunks.append(np.ascontiguousarray(chunk))

    in_maps = []
    for c in range(n_cores):
        featT_c = np.ascontiguousarray(
            features[c * npc : (c + 1) * npc, :].T.astype(np.float16)
        )
        im = {
            "featT": featT_c,
            "W1": W1_16,
            "W2": W2_16,
            "bsum": bsum,
            "idx16": per_core[c]["idx16"],
            "S8": per_core[c]["S8"],
        }
        for j in range(NCHUNKS):
            im[f"feat8_{j}"] = feat8_chunks[j]
        in_maps.append(im)

    res = bass_utils.run_bass_kernel_spmd(nc, in_maps, core_ids=list(range(n_cores)))
    _LAST_RESULTS = res
    out = np.concatenate(
        [r["outT"].T.astype(np.float32) for r in res.results], axis=0
    )
    return np.ascontiguousarray(out)


def kernel(rows, cols, vals, features, W1, b1, W2, b2):
    return _run(rows, cols, vals, features, W1, b1, W2, b2, N_NODES, N_CORES)
